# revision 33
# baseline (speedup 1.0000x reference)
"""CrossNet forward as a Trainium2 Bass/Tile kernel, data-parallel over 8 cores.

Math: the CrossNet layer stack
    x_{l+1} = x0 * (x_l . w_l) + b_l + x_l            (l = 0..3)
collapses in closed form.  Writing x_l = x0 * alpha_l[b] + beta_l[d]:
    p_l[b]     = sum_d x0[b,d] w_l[d]                 (4 projections of x0)
    alpha_0    = 1,   alpha_{l+1} = alpha_l * (1 + p_l) + c_l
    beta_{l+1} = beta_l + b_l,  c_l = beta_l . w_l    (host-computable scalars)
    out        = x0 * alpha_4[b] + beta_4[d]

The host rounds x to fp16.  Per 1024-row supertile the device transposes the
eight 128-row chunks to get features onto partitions for the projection
matmuls; transposes are split between the PE (pairs of chunks packed into
fp32 words, moved by bit-exact fp32-dtype transposes) and the DMA XBAR
transpose engine (raw fp16 chunks), because the PE alone is the throughput
wall.  Then: 8 fp16 [128d,128b]^T @ [128d,4] projection matmuls, a tiny f32
DVE recurrence for alpha, and one broadcast multiply fp16(x)*alpha -> f32.
End-to-end error ~6e-4 (fp16 x quantization + fp16 projections).
"""

import numpy as np

B = 500_000
D = 128
L = 4
N_CORES = 8
ROWS = B // N_CORES          # 62500 rows per core
G = 8                        # 128-row chunks per supertile
SUP = 128 * G                # 1024 rows per supertile
NSUP = ROWS // SUP           # 61 full supertiles
REM = ROWS - NSUP * SUP      # 36 remainder rows

# Chunks per supertile transposed by the DMA XBAR engine (rest go through
# packed PE transposes). The XBAR path measured ~1.5us of HWDGE ring time per
# chunk on this runtime -- keep it off.
DM = 0
NPAIR = (G - DM) // 2
PE_COLS = 2 * NPAIR * D      # fp16 columns holding packed pairs
DM_COLS = DM * D             # fp16 columns holding contiguous DMAT chunks
# Dtype for the packed pair transposes. float32's LOW_HIGH mode routes the
# two 16-bit halves bit-exactly; float32r was measured to CORRUPT packed fp16
# patterns on HW (rel err ~3.5) -- do not use it here.
TDT = "float32"

_CACHE: dict = {}

# test.py can read run metadata (exec_time_ns etc.) from here after a call.
LAST_RESULTS = None


def _build(cs, has_bias):
    import concourse.tile as tile
    from concourse import bacc, mybir

    f32 = mybir.dt.float32
    f16 = mybir.dt.float16
    tdt = getattr(mybir.dt, TDT)
    mult = mybir.AluOpType.mult
    add = mybir.AluOpType.add

    nc = bacc.Bacc(
        "TRN2",
        target_bir_lowering=False,
        debug=False,
        enable_asserts=False,
        num_devices=N_CORES,
    )
    # xp: host-prepared fp16 supertiles. Free layout per partition:
    #   [pair j=0..NPAIR-1 interleaved (j, d, q) | chunk NPAIR*2+k contiguous]
    # with chunk g = 2j+q for the packed part.
    xp = nc.dram_tensor("xp", [NSUP, 128, G * D // 2], tdt, kind="ExternalInput").ap()
    xrem = None
    if REM:
        xrem = nc.dram_tensor("xrem", [REM, D], f16, kind="ExternalInput").ap()
    w = nc.dram_tensor("w", [D, L], f16, kind="ExternalInput").ap()
    ident = nc.dram_tensor("ident", [128, 128], f16, kind="ExternalInput").ap()
    ident32 = nc.dram_tensor("ident32", [128, 128], tdt, kind="ExternalInput").ap()
    bb = None
    if has_bias:
        bb = nc.dram_tensor("bb", [128, D], f32, kind="ExternalInput").ap()
    # fp16 output halves store traffic; the host upcasts to f32. Output
    # values are fp16(x)*alpha products, so this costs ~2.4e-4 extra.
    out = nc.dram_tensor("out", [ROWS, D], f16, kind="ExternalOutput").ap()

    # Store view: row = s*1024 + p*8 + g, free (g d) contiguous per partition.
    ov = out[0 : NSUP * SUP, :].rearrange("(s p g) d -> s p (g d)", p=128, g=G)

    with tile.TileContext(nc) as tc:
        with (
            tc.tile_pool(name="consts", bufs=1) as cpool,
            tc.tile_pool(name="xin", bufs=24) as xpool,
            tc.tile_pool(name="xt", bufs=8) as xtpool,
            tc.tile_pool(name="xtd", bufs=2 * DM + 4) as xtdpool,
            tc.tile_pool(name="xtps", bufs=2, space="PSUM") as tps_pool,
            tc.tile_pool(name="ptps", bufs=4, space="PSUM") as pps_pool,
            tc.tile_pool(name="small", bufs=16) as spool,
            tc.tile_pool(name="outp", bufs=12) as opool,
        ):
            ident_sb = cpool.tile([128, 128], f16, tag="ident")
            nc.sync.dma_start(ident_sb[:], ident)
            ident32_sb = cpool.tile([128, 128], tdt, tag="ident32")
            nc.sync.dma_start(ident32_sb[:], ident32)
            w_sb = cpool.tile([D, L], f16, tag="w")
            nc.sync.dma_start(w_sb[:], w)
            bb_sb = None
            if has_bias:
                bb_sb = cpool.tile([128, D], f32, tag="bb")
                nc.sync.dma_start(bb_sb[:], bb)

            def alpha_from_pt(pt_ps, p_cnt, g_cnt):
                # q = 1 + p, then alpha = Horner chain over the 4 layers.
                q_sb = spool.tile([p_cnt, L * g_cnt], f32, tag="q")
                nc.scalar.add(q_sb[:], pt_ps[:], 1.0)
                qv = q_sb[:].rearrange("p (g l) -> p g l", l=L)
                if has_bias:
                    a = spool.tile([p_cnt, g_cnt], f32, tag="a0")
                    # c_0 == 0 always (beta_0 = 0)
                    nc.vector.tensor_copy(a[:], qv[:, :, 0])
                    for l in range(1, L):
                        t = spool.tile([p_cnt, g_cnt], f32, tag=f"a{l}")
                        nc.vector.tensor_mul(t[:], a[:], qv[:, :, l])
                        if cs[l] != 0.0:
                            t2 = spool.tile([p_cnt, g_cnt], f32, tag=f"ac{l}")
                            nc.vector.tensor_scalar_add(t2[:], t[:], float(cs[l]))
                            t = t2
                        a = t
                else:
                    # t[g, u] = q[g, 2u] * q[g, 2u+1], then a = t[:,0]*t[:,1]
                    qp = q_sb[:].rearrange("p (g u l) -> p g u l", u=2, l=2)
                    t = spool.tile([p_cnt, 2 * g_cnt], f32, tag="a1")
                    tv = t[:].rearrange("p (g u) -> p g u", u=2)
                    nc.vector.tensor_mul(tv, qp[:, :, :, 0], qp[:, :, :, 1])
                    a = spool.tile([p_cnt, g_cnt], f32, tag="a3")
                    nc.vector.tensor_mul(a[:], tv[:, :, 0], tv[:, :, 1])
                return a

            def block_packed(s):
                xp_sb = xpool.tile([128, G * D // 2], tdt, tag="x")
                nc.sync.dma_start(xp_sb[:], xp[s])
                xp16 = xp_sb[:].bitcast(f16)  # [128, G*D]
                pt_ps = pps_pool.tile([128, L * G], f32, tag="pt")

                # --- PE-transposed packed pairs (chunks 0 .. 2*NPAIR-1) ---
                xp32 = xp_sb[:]  # [128, G*D/2] packed words
                # Pair j -> PSUM bank j%2 so the ACT copy of one bank overlaps
                # PE writes to the other.
                xt_ps = tps_pool.tile([128, 1024], tdt, tag="xtps")
                xt_sb = xtpool.tile([128, NPAIR * D], tdt, tag="xt")
                halfp = (NPAIR + 1) // 2
                pso = lambda j: (j * D) if j < halfp else (512 + (j - halfp) * D)

                for j in range(halfp):
                    nc.tensor.transpose(
                        xt_ps[:, pso(j) : pso(j) + D],
                        xp32[:, j * D : (j + 1) * D],
                        ident32_sb[:],
                    )
                nc.scalar.copy(xt_sb[:, : halfp * D], xt_ps[:, : halfp * D])
                for j in range(halfp, NPAIR):
                    nc.tensor.transpose(
                        xt_ps[:, pso(j) : pso(j) + D],
                        xp32[:, j * D : (j + 1) * D],
                        ident32_sb[:],
                    )
                if NPAIR > halfp:
                    nc.scalar.copy(
                        xt_sb[:, halfp * D :],
                        xt_ps[:, 512 : 512 + (NPAIR - halfp) * D],
                    )

                # --- DMA-XBAR-transposed chunks (2*NPAIR .. G-1) ---
                xt_d = []
                for k in range(DM):
                    t = xtdpool.tile([128, D], f16, tag=f"xtd{k % 2}")
                    ring = nc.sync if k % 2 == 0 else nc.scalar
                    c0 = PE_COLS + k * D
                    ring.dma_start_transpose(t[:], xp16[:, c0 : c0 + D])
                    xt_d.append(t)

                # --- projection matmuls ---
                xt16 = xt_sb[:].bitcast(f16).rearrange("d (j b q) -> d j b q", b=D, q=2)
                for g in range(2 * NPAIR):
                    j, qq = g // 2, g % 2
                    nc.tensor.matmul(
                        pt_ps[:, g * L : (g + 1) * L],
                        lhsT=xt16[:, j, :, qq],
                        rhs=w_sb[:],
                        start=True,
                        stop=True,
                    )
                for k in range(DM):
                    g = 2 * NPAIR + k
                    nc.tensor.matmul(
                        pt_ps[:, g * L : (g + 1) * L],
                        lhsT=xt_d[k][:],
                        rhs=w_sb[:],
                        start=True,
                        stop=True,
                    )

                a = alpha_from_pt(pt_ps, 128, G)

                # out_sb uses plain chunk-major (g d) layout so the store DMA
                # is a 2-dim contiguous transfer.
                out_sb = opool.tile([128, G * D], f16, tag="o")
                o_pk = out_sb[:, :PE_COLS].rearrange("p (j q d) -> p j q d", q=2, d=D)
                x_pk = xp16[:, :PE_COLS].rearrange("p (j d q) -> p j q d", d=D, q=2)
                a_pk = a[:, : 2 * NPAIR].rearrange("p (j q) -> p j q", q=2).to_broadcast(
                    [128, NPAIR, 2, D]
                )
                if DM:
                    o_dm = out_sb[:, PE_COLS:].rearrange("p (k d) -> p k d", d=D)
                    x_dm = xp16[:, PE_COLS:].rearrange("p (k d) -> p k d", d=D)
                    a_dm = a[:, 2 * NPAIR :].to_broadcast([128, DM, D])
                if has_bias:
                    t_sb = opool.tile([128, G * D], f32, tag="t")
                    t_pk = t_sb[:, :PE_COLS].rearrange("p (j q d) -> p j q d", q=2, d=D)
                    nc.vector.tensor_mul(t_pk, x_pk, a_pk)
                    if DM:
                        t_dm = t_sb[:, PE_COLS:].rearrange("p (k d) -> p k d", d=D)
                        nc.vector.tensor_mul(t_dm, x_dm, a_dm)
                    for g in range(G):
                        nc.vector.tensor_add(
                            out_sb[:, g * D : (g + 1) * D],
                            t_sb[:, g * D : (g + 1) * D],
                            bb_sb[:, :],
                        )
                else:
                    nc.vector.tensor_mul(o_pk, x_pk, a_pk)
                    if DM:
                        nc.vector.tensor_mul(o_dm, x_dm, a_dm)
                nc.sync.dma_start(ov[s], out_sb[:])

            def block_rem():
                p_cnt = REM
                x_sb = xpool.tile([p_cnt, D], f16, tag="x")
                nc.sync.dma_start(x_sb[:], xrem)
                xt_ps = tps_pool.tile([128, p_cnt], f16, tag="xtps")
                xt_sb = xtpool.tile([128, p_cnt], f16, tag="xt")
                pt_ps = pps_pool.tile([p_cnt, L], f32, tag="pt")
                nc.tensor.transpose(xt_ps[:], x_sb[:], ident_sb[:p_cnt, :p_cnt])
                nc.scalar.copy(xt_sb[:], xt_ps[:])
                nc.tensor.matmul(
                    pt_ps[:], lhsT=xt_sb[:], rhs=w_sb[:], start=True, stop=True
                )
                a = alpha_from_pt(pt_ps, p_cnt, 1)
                out_sb = opool.tile([p_cnt, D], f16, tag="or")
                if has_bias:
                    nc.vector.scalar_tensor_tensor(
                        out_sb[:], x_sb[:], a[:, 0:1], bb_sb[:p_cnt, :],
                        op0=mult, op1=add,
                    )
                else:
                    nc.vector.tensor_mul(
                        out_sb[:].rearrange("p (u d) -> p u d", u=1),
                        x_sb[:].rearrange("p (u d) -> p u d", u=1),
                        a[:].to_broadcast([p_cnt, 1, D]),
                    )
                nc.sync.dma_start(out[NSUP * SUP :, :], out_sb[:])

            for s in range(NSUP):
                block_packed(s)
            if REM:
                block_rem()

    nc.compile()
    return nc


def _pack_shard(xs):
    # xs: [ROWS, D] float32 -> fp16 [NSUP, 128, G*D]:
    #   cols [0 : PE_COLS)      pairs (j, d, q): chunk g = 2j+q
    #   cols [PE_COLS : G*D)    chunks 2*NPAIR..G-1 contiguous (k, d)
    x16 = xs[: NSUP * SUP].astype(np.float16).reshape(NSUP, 128, G, D)
    parts = []
    if NPAIR:
        pk = x16[:, :, : 2 * NPAIR, :].reshape(NSUP, 128, NPAIR, 2, D)
        parts.append(
            np.ascontiguousarray(pk.transpose(0, 1, 2, 4, 3)).reshape(NSUP, 128, -1)
        )
    if DM:
        parts.append(
            np.ascontiguousarray(x16[:, :, 2 * NPAIR :, :]).reshape(NSUP, 128, -1)
        )
    pk = np.ascontiguousarray(np.concatenate(parts, axis=2))
    return pk.view(np.float32)


def kernel(inputs, kernels, biases):
    global LAST_RESULTS
    from concourse.bass_utils import run_bass_kernel_spmd

    x = np.ascontiguousarray(np.asarray(inputs), dtype=np.float32)
    assert x.shape == (B, D), x.shape
    kern = np.asarray(kernels, dtype=np.float32).reshape(L, D)
    bias = np.asarray(biases, dtype=np.float32).reshape(L, D)

    W = np.ascontiguousarray(kern.T)  # [D, L]
    has_bias = bool(np.any(bias))
    cs = []
    beta = np.zeros(D, dtype=np.float32)
    for l in range(L):
        cs.append(float(np.dot(beta.astype(np.float64), kern[l].astype(np.float64))))
        beta = beta + bias[l]

    key = (has_bias, tuple(cs) if has_bias else None)
    nc = _CACHE.get(key)
    if nc is None:
        nc = _build(cs, has_bias)
        _CACHE[key] = nc

    bbcast = np.ascontiguousarray(np.broadcast_to(beta, (128, D)), dtype=np.float32)
    in_maps = []
    for i in range(N_CORES):
        xs = x[i * ROWS : (i + 1) * ROWS]
        m = {
            "xp": _pack_shard(xs),
            "w": W.astype(np.float16),
            "ident": np.eye(128, dtype=np.float16),
            "ident32": np.eye(128, dtype=np.float32),
        }
        if REM:
            m["xrem"] = xs[NSUP * SUP :].astype(np.float16)
        if has_bias:
            m["bb"] = bbcast
        in_maps.append(m)

    res = run_bass_kernel_spmd(nc, in_maps, core_ids=list(range(N_CORES)))
    LAST_RESULTS = res
    return np.concatenate(
        [res.results[i]["out"] for i in range(N_CORES)], axis=0
    ).astype(np.float32)


# revision 34
# speedup vs baseline: 1.0498x; 1.0498x over previous
"""CrossNet forward as a Trainium2 Bass/Tile kernel, data-parallel over 8 cores.

Math: the CrossNet layer stack
    x_{l+1} = x0 * (x_l . w_l) + b_l + x_l            (l = 0..3)
collapses in closed form.  Writing x_l = x0 * alpha_l[b] + beta_l[d]:
    p_l[b]     = sum_d x0[b,d] w_l[d]                 (4 projections of x0)
    alpha_0    = 1,   alpha_{l+1} = alpha_l * (1 + p_l) + c_l
    beta_{l+1} = beta_l + b_l,  c_l = beta_l . w_l    (host-computable scalars)
    out        = x0 * alpha_4[b] + beta_4[d]

The host rounds x to fp16.  Per 1024-row supertile the device transposes the
eight 128-row chunks to get features onto partitions for the projection
matmuls; transposes are split between the PE (pairs of chunks packed into
fp32 words, moved by bit-exact fp32-dtype transposes) and the DMA XBAR
transpose engine (raw fp16 chunks), because the PE alone is the throughput
wall.  Then: 8 fp16 [128d,128b]^T @ [128d,4] projection matmuls, a tiny f32
DVE recurrence for alpha, and one broadcast multiply fp16(x)*alpha -> f32.
End-to-end error ~6e-4 (fp16 x quantization + fp16 projections).
"""

import numpy as np

B = 500_000
D = 128
L = 4
N_CORES = 8
ROWS = B // N_CORES          # 62500 rows per core
G = 8                        # 128-row chunks per supertile
SUP = 128 * G                # 1024 rows per supertile
NSUP = ROWS // SUP           # 61 full supertiles
REM = ROWS - NSUP * SUP      # 36 remainder rows

# Chunks per supertile transposed by the DMA XBAR engine (rest go through
# packed PE transposes). The XBAR path measured ~1.5us of HWDGE ring time per
# chunk on this runtime -- keep it off.
DM = 0
NPAIR = (G - DM) // 2
PE_COLS = 2 * NPAIR * D      # fp16 columns holding packed pairs
DM_COLS = DM * D             # fp16 columns holding contiguous DMAT chunks
# Dtype for the packed pair transposes. float32's LOW_HIGH mode routes the
# two 16-bit halves bit-exactly; float32r was measured to CORRUPT packed fp16
# patterns on HW (rel err ~3.5) -- do not use it here.
TDT = "float32"

_CACHE: dict = {}

# test.py can read run metadata (exec_time_ns etc.) from here after a call.
LAST_RESULTS = None


def _build(cs, has_bias):
    import concourse.tile as tile
    from concourse import bacc, mybir

    f32 = mybir.dt.float32
    f16 = mybir.dt.float16
    tdt = getattr(mybir.dt, TDT)
    mult = mybir.AluOpType.mult
    add = mybir.AluOpType.add

    nc = bacc.Bacc(
        "TRN2",
        target_bir_lowering=False,
        debug=False,
        enable_asserts=False,
        num_devices=N_CORES,
    )
    # xp: host-prepared fp16 supertiles. Free layout per partition:
    #   [pair j=0..NPAIR-1 interleaved (j, d, q) | chunk NPAIR*2+k contiguous]
    # with chunk g = 2j+q for the packed part.
    xp = nc.dram_tensor("xp", [NSUP, 128, G * D // 2], tdt, kind="ExternalInput").ap()
    xrem = None
    if REM:
        xrem = nc.dram_tensor("xrem", [REM, D], f16, kind="ExternalInput").ap()
    w = nc.dram_tensor("w", [D, L], f16, kind="ExternalInput").ap()
    ident = nc.dram_tensor("ident", [128, 128], f16, kind="ExternalInput").ap()
    ident32 = nc.dram_tensor("ident32", [128, 128], tdt, kind="ExternalInput").ap()
    bb = None
    if has_bias:
        bb = nc.dram_tensor("bb", [128, D], f32, kind="ExternalInput").ap()
    # fp16 output halves store traffic; the host upcasts to f32. Output
    # values are fp16(x)*alpha products, so this costs ~2.4e-4 extra.
    out = nc.dram_tensor("out", [ROWS, D], f16, kind="ExternalOutput").ap()

    # Store view: row = s*1024 + p*8 + g, free (g d) contiguous per partition.
    ov = out[0 : NSUP * SUP, :].rearrange("(s p g) d -> s p (g d)", p=128, g=G)

    with tile.TileContext(nc) as tc:
        with (
            tc.tile_pool(name="consts", bufs=1) as cpool,
            tc.tile_pool(name="xin", bufs=24) as xpool,
            tc.tile_pool(name="xt", bufs=8) as xtpool,
            tc.tile_pool(name="xtd", bufs=2 * DM + 4) as xtdpool,
            tc.tile_pool(name="xtps", bufs=3, space="PSUM") as tps_pool,
            tc.tile_pool(name="ptps", bufs=2, space="PSUM") as pps_pool,
            tc.tile_pool(name="small", bufs=16) as spool,
            tc.tile_pool(name="outp", bufs=12) as opool,
        ):
            ident_sb = cpool.tile([128, 128], f16, tag="ident")
            nc.sync.dma_start(ident_sb[:], ident)
            ident32_sb = cpool.tile([128, 128], tdt, tag="ident32")
            nc.sync.dma_start(ident32_sb[:], ident32)
            w_sb = cpool.tile([D, L], f16, tag="w")
            nc.sync.dma_start(w_sb[:], w)
            bb_sb = None
            if has_bias:
                bb_sb = cpool.tile([128, D], f32, tag="bb")
                nc.sync.dma_start(bb_sb[:], bb)

            def alpha_from_pt(pt_ps, p_cnt, g_cnt):
                # q = 1 + p, then alpha = Horner chain over the 4 layers.
                q_sb = spool.tile([p_cnt, L * g_cnt], f32, tag="q")
                nc.vector.tensor_scalar_add(q_sb[:], pt_ps[:], 1.0)
                qv = q_sb[:].rearrange("p (g l) -> p g l", l=L)
                if has_bias:
                    a = spool.tile([p_cnt, g_cnt], f32, tag="a0")
                    # c_0 == 0 always (beta_0 = 0)
                    nc.vector.tensor_copy(a[:], qv[:, :, 0])
                    for l in range(1, L):
                        t = spool.tile([p_cnt, g_cnt], f32, tag=f"a{l}")
                        nc.vector.tensor_mul(t[:], a[:], qv[:, :, l])
                        if cs[l] != 0.0:
                            t2 = spool.tile([p_cnt, g_cnt], f32, tag=f"ac{l}")
                            nc.vector.tensor_scalar_add(t2[:], t[:], float(cs[l]))
                            t = t2
                        a = t
                else:
                    # t[g, u] = q[g, 2u] * q[g, 2u+1], then a = t[:,0]*t[:,1]
                    qp = q_sb[:].rearrange("p (g u l) -> p g u l", u=2, l=2)
                    t = spool.tile([p_cnt, 2 * g_cnt], f32, tag="a1")
                    tv = t[:].rearrange("p (g u) -> p g u", u=2)
                    nc.vector.tensor_mul(tv, qp[:, :, :, 0], qp[:, :, :, 1])
                    a = spool.tile([p_cnt, g_cnt], f32, tag="a3")
                    nc.vector.tensor_mul(a[:], tv[:, :, 0], tv[:, :, 1])
                return a

            def block_packed(s):
                xp_sb = xpool.tile([128, G * D // 2], tdt, tag="x")
                nc.sync.dma_start(xp_sb[:], xp[s])
                xp16 = xp_sb[:].bitcast(f16)  # [128, G*D]
                pt_ps = pps_pool.tile([128, L * G], f32, tag="pt")

                # --- PE-transposed packed pairs (chunks 0 .. 2*NPAIR-1) ---
                xp32 = xp_sb[:]  # [128, G*D/2] packed words
                # Pair j -> PSUM bank j%2 so the ACT copy of one bank overlaps
                # PE writes to the other.
                xt_ps = tps_pool.tile([128, 1024], tdt, tag="xtps")
                xt_sb = xtpool.tile([128, NPAIR * D], tdt, tag="xt")
                halfp = (NPAIR + 1) // 2
                pso = lambda j: (j * D) if j < halfp else (512 + (j - halfp) * D)

                for j in range(halfp):
                    nc.tensor.transpose(
                        xt_ps[:, pso(j) : pso(j) + D],
                        xp32[:, j * D : (j + 1) * D],
                        ident32_sb[:],
                    )
                nc.scalar.copy(xt_sb[:, : halfp * D], xt_ps[:, : halfp * D])
                for j in range(halfp, NPAIR):
                    nc.tensor.transpose(
                        xt_ps[:, pso(j) : pso(j) + D],
                        xp32[:, j * D : (j + 1) * D],
                        ident32_sb[:],
                    )
                if NPAIR > halfp:
                    nc.scalar.copy(
                        xt_sb[:, halfp * D :],
                        xt_ps[:, 512 : 512 + (NPAIR - halfp) * D],
                    )

                # --- DMA-XBAR-transposed chunks (2*NPAIR .. G-1) ---
                xt_d = []
                for k in range(DM):
                    t = xtdpool.tile([128, D], f16, tag=f"xtd{k % 2}")
                    ring = nc.sync if k % 2 == 0 else nc.scalar
                    c0 = PE_COLS + k * D
                    ring.dma_start_transpose(t[:], xp16[:, c0 : c0 + D])
                    xt_d.append(t)

                # --- projection matmuls ---
                xt16 = xt_sb[:].bitcast(f16).rearrange("d (j b q) -> d j b q", b=D, q=2)
                for g in range(2 * NPAIR):
                    j, qq = g // 2, g % 2
                    nc.tensor.matmul(
                        pt_ps[:, g * L : (g + 1) * L],
                        lhsT=xt16[:, j, :, qq],
                        rhs=w_sb[:],
                        start=True,
                        stop=True,
                    )
                for k in range(DM):
                    g = 2 * NPAIR + k
                    nc.tensor.matmul(
                        pt_ps[:, g * L : (g + 1) * L],
                        lhsT=xt_d[k][:],
                        rhs=w_sb[:],
                        start=True,
                        stop=True,
                    )

                a = alpha_from_pt(pt_ps, 128, G)

                # out_sb uses plain chunk-major (g d) layout so the store DMA
                # is a 2-dim contiguous transfer.
                out_sb = opool.tile([128, G * D], f16, tag="o")
                o_pk = out_sb[:, :PE_COLS].rearrange("p (j q d) -> p j q d", q=2, d=D)
                x_pk = xp16[:, :PE_COLS].rearrange("p (j d q) -> p j q d", d=D, q=2)
                a_pk = a[:, : 2 * NPAIR].rearrange("p (j q) -> p j q", q=2).to_broadcast(
                    [128, NPAIR, 2, D]
                )
                if DM:
                    o_dm = out_sb[:, PE_COLS:].rearrange("p (k d) -> p k d", d=D)
                    x_dm = xp16[:, PE_COLS:].rearrange("p (k d) -> p k d", d=D)
                    a_dm = a[:, 2 * NPAIR :].to_broadcast([128, DM, D])
                if has_bias:
                    t_sb = opool.tile([128, G * D], f32, tag="t")
                    t_pk = t_sb[:, :PE_COLS].rearrange("p (j q d) -> p j q d", q=2, d=D)
                    nc.vector.tensor_mul(t_pk, x_pk, a_pk)
                    if DM:
                        t_dm = t_sb[:, PE_COLS:].rearrange("p (k d) -> p k d", d=D)
                        nc.vector.tensor_mul(t_dm, x_dm, a_dm)
                    for g in range(G):
                        nc.vector.tensor_add(
                            out_sb[:, g * D : (g + 1) * D],
                            t_sb[:, g * D : (g + 1) * D],
                            bb_sb[:, :],
                        )
                else:
                    nc.vector.tensor_mul(o_pk, x_pk, a_pk)
                    if DM:
                        nc.vector.tensor_mul(o_dm, x_dm, a_dm)
                nc.gpsimd.dma_start(ov[s], out_sb[:])

            def block_rem():
                p_cnt = REM
                x_sb = xpool.tile([p_cnt, D], f16, tag="x")
                nc.sync.dma_start(x_sb[:], xrem)
                xt_ps = tps_pool.tile([128, p_cnt], f16, tag="xtps")
                xt_sb = xtpool.tile([128, p_cnt], f16, tag="xt")
                pt_ps = pps_pool.tile([p_cnt, L], f32, tag="pt")
                nc.tensor.transpose(xt_ps[:], x_sb[:], ident_sb[:p_cnt, :p_cnt])
                nc.scalar.copy(xt_sb[:], xt_ps[:])
                nc.tensor.matmul(
                    pt_ps[:], lhsT=xt_sb[:], rhs=w_sb[:], start=True, stop=True
                )
                a = alpha_from_pt(pt_ps, p_cnt, 1)
                out_sb = opool.tile([p_cnt, D], f16, tag="or")
                if has_bias:
                    nc.vector.scalar_tensor_tensor(
                        out_sb[:], x_sb[:], a[:, 0:1], bb_sb[:p_cnt, :],
                        op0=mult, op1=add,
                    )
                else:
                    nc.vector.tensor_mul(
                        out_sb[:].rearrange("p (u d) -> p u d", u=1),
                        x_sb[:].rearrange("p (u d) -> p u d", u=1),
                        a[:].to_broadcast([p_cnt, 1, D]),
                    )
                nc.gpsimd.dma_start(out[NSUP * SUP :, :], out_sb[:])

            for s in range(NSUP):
                block_packed(s)
            if REM:
                block_rem()

    nc.compile()
    return nc


def _pack_shard(xs):
    # xs: [ROWS, D] float32 -> fp16 [NSUP, 128, G*D]:
    #   cols [0 : PE_COLS)      pairs (j, d, q): chunk g = 2j+q
    #   cols [PE_COLS : G*D)    chunks 2*NPAIR..G-1 contiguous (k, d)
    x16 = xs[: NSUP * SUP].astype(np.float16).reshape(NSUP, 128, G, D)
    parts = []
    if NPAIR:
        pk = x16[:, :, : 2 * NPAIR, :].reshape(NSUP, 128, NPAIR, 2, D)
        parts.append(
            np.ascontiguousarray(pk.transpose(0, 1, 2, 4, 3)).reshape(NSUP, 128, -1)
        )
    if DM:
        parts.append(
            np.ascontiguousarray(x16[:, :, 2 * NPAIR :, :]).reshape(NSUP, 128, -1)
        )
    pk = np.ascontiguousarray(np.concatenate(parts, axis=2))
    return pk.view(np.float32)


def kernel(inputs, kernels, biases):
    global LAST_RESULTS
    from concourse.bass_utils import run_bass_kernel_spmd

    x = np.ascontiguousarray(np.asarray(inputs), dtype=np.float32)
    assert x.shape == (B, D), x.shape
    kern = np.asarray(kernels, dtype=np.float32).reshape(L, D)
    bias = np.asarray(biases, dtype=np.float32).reshape(L, D)

    W = np.ascontiguousarray(kern.T)  # [D, L]
    has_bias = bool(np.any(bias))
    cs = []
    beta = np.zeros(D, dtype=np.float32)
    for l in range(L):
        cs.append(float(np.dot(beta.astype(np.float64), kern[l].astype(np.float64))))
        beta = beta + bias[l]

    key = (has_bias, tuple(cs) if has_bias else None)
    nc = _CACHE.get(key)
    if nc is None:
        nc = _build(cs, has_bias)
        _CACHE[key] = nc

    bbcast = np.ascontiguousarray(np.broadcast_to(beta, (128, D)), dtype=np.float32)
    in_maps = []
    for i in range(N_CORES):
        xs = x[i * ROWS : (i + 1) * ROWS]
        m = {
            "xp": _pack_shard(xs),
            "w": W.astype(np.float16),
            "ident": np.eye(128, dtype=np.float16),
            "ident32": np.eye(128, dtype=np.float32),
        }
        if REM:
            m["xrem"] = xs[NSUP * SUP :].astype(np.float16)
        if has_bias:
            m["bb"] = bbcast
        in_maps.append(m)

    res = run_bass_kernel_spmd(nc, in_maps, core_ids=list(range(N_CORES)))
    LAST_RESULTS = res
    return np.concatenate(
        [res.results[i]["out"] for i in range(N_CORES)], axis=0
    ).astype(np.float32)


# revision 35
# speedup vs baseline: 1.0735x; 1.0226x over previous
"""CrossNet forward as a Trainium2 Bass/Tile kernel, data-parallel over 8 cores.

Math: the CrossNet layer stack
    x_{l+1} = x0 * (x_l . w_l) + b_l + x_l            (l = 0..3)
collapses in closed form.  Writing x_l = x0 * alpha_l[b] + beta_l[d]:
    p_l[b]     = sum_d x0[b,d] w_l[d]                 (4 projections of x0)
    alpha_0    = 1,   alpha_{l+1} = alpha_l * (1 + p_l) + c_l
    beta_{l+1} = beta_l + b_l,  c_l = beta_l . w_l    (host-computable scalars)
    out        = x0 * alpha_4[b] + beta_4[d]

The host rounds x to fp16.  Per 1024-row supertile the device transposes the
eight 128-row chunks to get features onto partitions for the projection
matmuls; transposes are split between the PE (pairs of chunks packed into
fp32 words, moved by bit-exact fp32-dtype transposes) and the DMA XBAR
transpose engine (raw fp16 chunks), because the PE alone is the throughput
wall.  Then: 8 fp16 [128d,128b]^T @ [128d,4] projection matmuls, a tiny f32
DVE recurrence for alpha, and one broadcast multiply fp16(x)*alpha -> f32.
End-to-end error ~6e-4 (fp16 x quantization + fp16 projections).
"""

import numpy as np

B = 500_000
D = 128
L = 4
N_CORES = 8
ROWS = B // N_CORES          # 62500 rows per core
G = 8                        # 128-row chunks per supertile
SUP = 128 * G                # 1024 rows per supertile
NSUP = ROWS // SUP           # 61 full supertiles
REM = ROWS - NSUP * SUP      # 36 remainder rows

# Chunks per supertile transposed by the DMA XBAR engine (rest go through
# packed PE transposes). The XBAR path measured ~1.5us of HWDGE ring time per
# chunk on this runtime -- keep it off.
DM = 0
NPAIR = (G - DM) // 2
PE_COLS = 2 * NPAIR * D      # fp16 columns holding packed pairs
DM_COLS = DM * D             # fp16 columns holding contiguous DMAT chunks
# Dtype for the packed pair transposes. float32's LOW_HIGH mode routes the
# two 16-bit halves bit-exactly; float32r was measured to CORRUPT packed fp16
# patterns on HW (rel err ~3.5) -- do not use it here.
TDT = "float32"

_CACHE: dict = {}

# test.py can read run metadata (exec_time_ns etc.) from here after a call.
LAST_RESULTS = None


def _build(cs, has_bias):
    import concourse.tile as tile
    from concourse import bacc, mybir

    f32 = mybir.dt.float32
    f16 = mybir.dt.float16
    tdt = getattr(mybir.dt, TDT)
    mult = mybir.AluOpType.mult
    add = mybir.AluOpType.add

    nc = bacc.Bacc(
        "TRN2",
        target_bir_lowering=False,
        debug=False,
        enable_asserts=False,
        num_devices=N_CORES,
    )
    # xp: host-prepared fp16 supertiles. Free layout per partition:
    #   [pair j=0..NPAIR-1 interleaved (j, d, q) | chunk NPAIR*2+k contiguous]
    # with chunk g = 2j+q for the packed part.
    xp = nc.dram_tensor("xp", [NSUP, 128, G * D // 2], tdt, kind="ExternalInput").ap()
    xrem = None
    if REM:
        xrem = nc.dram_tensor("xrem", [REM, D], f16, kind="ExternalInput").ap()
    w = nc.dram_tensor("w", [D, L], f16, kind="ExternalInput").ap()
    ident = nc.dram_tensor("ident", [128, 128], f16, kind="ExternalInput").ap()
    ident32 = nc.dram_tensor("ident32", [128, 128], tdt, kind="ExternalInput").ap()
    bb = None
    if has_bias:
        bb = nc.dram_tensor("bb", [128, D], f32, kind="ExternalInput").ap()
    # fp16 output halves store traffic; the host upcasts to f32. Output
    # values are fp16(x)*alpha products, so this costs ~2.4e-4 extra.
    out = nc.dram_tensor("out", [ROWS, D], f16, kind="ExternalOutput").ap()

    # Store view: row = s*1024 + p*8 + g, free (g d) contiguous per partition.
    ov = out[0 : NSUP * SUP, :].rearrange("(s p g) d -> s p (g d)", p=128, g=G)

    with tile.TileContext(nc) as tc:
        with (
            tc.tile_pool(name="consts", bufs=1) as cpool,
            tc.tile_pool(name="xin", bufs=24) as xpool,
            tc.tile_pool(name="xt", bufs=8) as xtpool,
            tc.tile_pool(name="xtd", bufs=2 * DM + 4) as xtdpool,
            tc.tile_pool(name="xtps", bufs=3, space="PSUM") as tps_pool,
            tc.tile_pool(name="ptps", bufs=2, space="PSUM") as pps_pool,
            tc.tile_pool(name="small", bufs=16) as spool,
            tc.tile_pool(name="outp", bufs=12) as opool,
        ):
            ident_sb = cpool.tile([128, 128], f16, tag="ident")
            nc.sync.dma_start(ident_sb[:], ident)
            ident32_sb = cpool.tile([128, 128], tdt, tag="ident32")
            nc.sync.dma_start(ident32_sb[:], ident32)
            w_sb = cpool.tile([D, L], f16, tag="w")
            nc.sync.dma_start(w_sb[:], w)
            bb_sb = None
            if has_bias:
                bb_sb = cpool.tile([128, D], f32, tag="bb")
                nc.sync.dma_start(bb_sb[:], bb)

            def alpha_from_pt(pt_ps, p_cnt, g_cnt):
                # q = 1 + p, then alpha = Horner chain over the 4 layers.
                q_sb = spool.tile([p_cnt, L * g_cnt], f32, tag="q")
                nc.vector.tensor_scalar_add(q_sb[:], pt_ps[:], 1.0)
                qv = q_sb[:].rearrange("p (g l) -> p g l", l=L)
                if has_bias:
                    a = spool.tile([p_cnt, g_cnt], f32, tag="a0")
                    # c_0 == 0 always (beta_0 = 0)
                    nc.vector.tensor_copy(a[:], qv[:, :, 0])
                    for l in range(1, L):
                        t = spool.tile([p_cnt, g_cnt], f32, tag=f"a{l}")
                        nc.vector.tensor_mul(t[:], a[:], qv[:, :, l])
                        if cs[l] != 0.0:
                            t2 = spool.tile([p_cnt, g_cnt], f32, tag=f"ac{l}")
                            nc.vector.tensor_scalar_add(t2[:], t[:], float(cs[l]))
                            t = t2
                        a = t
                else:
                    # t[g, u] = q[g, 2u] * q[g, 2u+1], then a = t[:,0]*t[:,1]
                    qp = q_sb[:].rearrange("p (g u l) -> p g u l", u=2, l=2)
                    t = spool.tile([p_cnt, 2 * g_cnt], f32, tag="a1")
                    tv = t[:].rearrange("p (g u) -> p g u", u=2)
                    nc.vector.tensor_mul(tv, qp[:, :, :, 0], qp[:, :, :, 1])
                    a = spool.tile([p_cnt, g_cnt], f32, tag="a3")
                    nc.vector.tensor_mul(a[:], tv[:, :, 0], tv[:, :, 1])
                return a

            def block_packed(s):
                xp_sb = xpool.tile([128, G * D // 2], tdt, tag="x")
                nc.sync.dma_start(xp_sb[:], xp[s])
                xp16 = xp_sb[:].bitcast(f16)  # [128, G*D]
                pt_ps = pps_pool.tile([128, L * G], f32, tag="pt")

                # --- PE-transposed packed pairs (chunks 0 .. 2*NPAIR-1) ---
                xp32 = xp_sb[:]  # [128, G*D/2] packed words
                # Pair j -> PSUM bank j%2 so the ACT copy of one bank overlaps
                # PE writes to the other.
                xt_ps = tps_pool.tile([128, 1024], tdt, tag="xtps")
                xt_sb = xtpool.tile([128, NPAIR * D], tdt, tag="xt")
                halfp = (NPAIR + 1) // 2
                pso = lambda j: (j * D) if j < halfp else (512 + (j - halfp) * D)

                for j in range(halfp):
                    nc.tensor.transpose(
                        xt_ps[:, pso(j) : pso(j) + D],
                        xp32[:, j * D : (j + 1) * D],
                        ident32_sb[:],
                    )
                nc.scalar.copy(xt_sb[:, : halfp * D], xt_ps[:, : halfp * D])
                for j in range(halfp, NPAIR):
                    nc.tensor.transpose(
                        xt_ps[:, pso(j) : pso(j) + D],
                        xp32[:, j * D : (j + 1) * D],
                        ident32_sb[:],
                    )
                if NPAIR > halfp:
                    nc.scalar.copy(
                        xt_sb[:, halfp * D :],
                        xt_ps[:, 512 : 512 + (NPAIR - halfp) * D],
                    )

                # --- DMA-XBAR-transposed chunks (2*NPAIR .. G-1) ---
                xt_d = []
                for k in range(DM):
                    t = xtdpool.tile([128, D], f16, tag=f"xtd{k % 2}")
                    ring = nc.sync if k % 2 == 0 else nc.scalar
                    c0 = PE_COLS + k * D
                    ring.dma_start_transpose(t[:], xp16[:, c0 : c0 + D])
                    xt_d.append(t)

                # --- projection matmuls ---
                xt16 = xt_sb[:].bitcast(f16).rearrange("d (j b q) -> d j b q", b=D, q=2)
                for g in range(2 * NPAIR):
                    j, qq = g // 2, g % 2
                    nc.tensor.matmul(
                        pt_ps[:, g * L : (g + 1) * L],
                        lhsT=xt16[:, j, :, qq],
                        rhs=w_sb[:],
                        start=True,
                        stop=True,
                    )
                for k in range(DM):
                    g = 2 * NPAIR + k
                    nc.tensor.matmul(
                        pt_ps[:, g * L : (g + 1) * L],
                        lhsT=xt_d[k][:],
                        rhs=w_sb[:],
                        start=True,
                        stop=True,
                    )

                a = alpha_from_pt(pt_ps, 128, G)
                return xp_sb, xp16, a

            def emit_final(s, xp_sb, xp16, a):
                # out_sb uses plain chunk-major (g d) layout so the store DMA
                # is a 2-dim contiguous transfer.
                out_sb = opool.tile([128, G * D], f16, tag="o")
                o_pk = out_sb[:, :PE_COLS].rearrange("p (j q d) -> p j q d", q=2, d=D)
                x_pk = xp16[:, :PE_COLS].rearrange("p (j d q) -> p j q d", d=D, q=2)
                a_pk = a[:, : 2 * NPAIR].rearrange("p (j q) -> p j q", q=2).to_broadcast(
                    [128, NPAIR, 2, D]
                )
                if DM:
                    o_dm = out_sb[:, PE_COLS:].rearrange("p (k d) -> p k d", d=D)
                    x_dm = xp16[:, PE_COLS:].rearrange("p (k d) -> p k d", d=D)
                    a_dm = a[:, 2 * NPAIR :].to_broadcast([128, DM, D])
                if has_bias:
                    t_sb = opool.tile([128, G * D], f32, tag="t")
                    t_pk = t_sb[:, :PE_COLS].rearrange("p (j q d) -> p j q d", q=2, d=D)
                    nc.vector.tensor_mul(t_pk, x_pk, a_pk)
                    if DM:
                        t_dm = t_sb[:, PE_COLS:].rearrange("p (k d) -> p k d", d=D)
                        nc.vector.tensor_mul(t_dm, x_dm, a_dm)
                    for g in range(G):
                        nc.vector.tensor_add(
                            out_sb[:, g * D : (g + 1) * D],
                            t_sb[:, g * D : (g + 1) * D],
                            bb_sb[:, :],
                        )
                else:
                    nc.vector.tensor_mul(o_pk, x_pk, a_pk)
                    if DM:
                        nc.vector.tensor_mul(o_dm, x_dm, a_dm)
                nc.gpsimd.dma_start(ov[s], out_sb[:])

            def block_rem():
                p_cnt = REM
                x_sb = xpool.tile([p_cnt, D], f16, tag="x")
                nc.sync.dma_start(x_sb[:], xrem)
                xt_ps = tps_pool.tile([128, p_cnt], f16, tag="xtps")
                xt_sb = xtpool.tile([128, p_cnt], f16, tag="xt")
                pt_ps = pps_pool.tile([p_cnt, L], f32, tag="pt")
                nc.tensor.transpose(xt_ps[:], x_sb[:], ident_sb[:p_cnt, :p_cnt])
                nc.scalar.copy(xt_sb[:], xt_ps[:])
                nc.tensor.matmul(
                    pt_ps[:], lhsT=xt_sb[:], rhs=w_sb[:], start=True, stop=True
                )
                a = alpha_from_pt(pt_ps, p_cnt, 1)
                out_sb = opool.tile([p_cnt, D], f16, tag="or")
                if has_bias:
                    nc.vector.scalar_tensor_tensor(
                        out_sb[:], x_sb[:], a[:, 0:1], bb_sb[:p_cnt, :],
                        op0=mult, op1=add,
                    )
                else:
                    nc.vector.tensor_mul(
                        out_sb[:].rearrange("p (u d) -> p u d", u=1),
                        x_sb[:].rearrange("p (u d) -> p u d", u=1),
                        a[:].to_broadcast([p_cnt, 1, D]),
                    )
                nc.gpsimd.dma_start(out[NSUP * SUP :, :], out_sb[:])

            pending = None
            for s in range(NSUP):
                front = block_packed(s)
                if pending is not None:
                    emit_final(s - 1, *pending)
                pending = front
            if pending is not None:
                emit_final(NSUP - 1, *pending)
            if REM:
                block_rem()

    nc.compile()
    return nc


def _pack_shard(xs):
    # xs: [ROWS, D] float32 -> fp16 [NSUP, 128, G*D]:
    #   cols [0 : PE_COLS)      pairs (j, d, q): chunk g = 2j+q
    #   cols [PE_COLS : G*D)    chunks 2*NPAIR..G-1 contiguous (k, d)
    x16 = xs[: NSUP * SUP].astype(np.float16).reshape(NSUP, 128, G, D)
    parts = []
    if NPAIR:
        pk = x16[:, :, : 2 * NPAIR, :].reshape(NSUP, 128, NPAIR, 2, D)
        parts.append(
            np.ascontiguousarray(pk.transpose(0, 1, 2, 4, 3)).reshape(NSUP, 128, -1)
        )
    if DM:
        parts.append(
            np.ascontiguousarray(x16[:, :, 2 * NPAIR :, :]).reshape(NSUP, 128, -1)
        )
    pk = np.ascontiguousarray(np.concatenate(parts, axis=2))
    return pk.view(np.float32)


def kernel(inputs, kernels, biases):
    global LAST_RESULTS
    from concourse.bass_utils import run_bass_kernel_spmd

    x = np.ascontiguousarray(np.asarray(inputs), dtype=np.float32)
    assert x.shape == (B, D), x.shape
    kern = np.asarray(kernels, dtype=np.float32).reshape(L, D)
    bias = np.asarray(biases, dtype=np.float32).reshape(L, D)

    W = np.ascontiguousarray(kern.T)  # [D, L]
    has_bias = bool(np.any(bias))
    cs = []
    beta = np.zeros(D, dtype=np.float32)
    for l in range(L):
        cs.append(float(np.dot(beta.astype(np.float64), kern[l].astype(np.float64))))
        beta = beta + bias[l]

    key = (has_bias, tuple(cs) if has_bias else None)
    nc = _CACHE.get(key)
    if nc is None:
        nc = _build(cs, has_bias)
        _CACHE[key] = nc

    bbcast = np.ascontiguousarray(np.broadcast_to(beta, (128, D)), dtype=np.float32)
    in_maps = []
    for i in range(N_CORES):
        xs = x[i * ROWS : (i + 1) * ROWS]
        m = {
            "xp": _pack_shard(xs),
            "w": W.astype(np.float16),
            "ident": np.eye(128, dtype=np.float16),
            "ident32": np.eye(128, dtype=np.float32),
        }
        if REM:
            m["xrem"] = xs[NSUP * SUP :].astype(np.float16)
        if has_bias:
            m["bb"] = bbcast
        in_maps.append(m)

    res = run_bass_kernel_spmd(nc, in_maps, core_ids=list(range(N_CORES)))
    LAST_RESULTS = res
    return np.concatenate(
        [res.results[i]["out"] for i in range(N_CORES)], axis=0
    ).astype(np.float32)
